# revision 11
# baseline (speedup 1.0000x reference)
"""Trainium2 Bass kernel for nn_MLPbiLm (bidirectional conv-window + highway MLP).

Reference computation (eval mode):
  padded = [left_pad(3), x, right_pad(3)]            # per sequence, [S+6, 128]
  left_inp[t]  = padded[t   : t+3]   (tokens t-3..t-1)  -> [384]
  right_inp[t] = padded[t+4 : t+7]   (tokens t+1..t+3)  -> [384]
  left  = highway2(left_inp @ lproj_w.T + lproj_b)
  right = highway2(right_inp @ rproj_w.T + rproj_b)
  out = concat([left, right], -1)                     # [B, S, 256]

Strategy:
  - Data-parallel over batch: 8 sequences per core on 8 NeuronCores.
  - Host prepares x^T in bf16 with padding baked in: xt[b] = [128(d), 4102(t)]
    so the window conv becomes 3 PSUM-accumulated matmuls over *shifted
    column views* of the same SBUF tile (contraction dim d on partitions).
  - All matmuls bf16 (N=512 free dim), PSUM fp32.
  - Work is software-pipelined across 16 units (= 8 seqs x 2 sides) in
    3 stages (conv / highway-0 / highway-1+store) so every engine always
    has independent work queued (engines execute in program order).
  - Elementwise balance: ACT does sigmoid (FD=2048), conv-bias evac and
    ~60% of relu evacs; DVE does the other relus (dual-op tensor_scalar)
    and the gate-combine tensor ops; Pool only does SWDGE cast-stores.
  - Output produced as [b, side, 128(h), 4096(t)] fp32; host transposes
    to [B, S, 256] during unshard.
"""

import numpy as np
import ml_dtypes

import concourse.bass as bass  # noqa: F401
import concourse.mybir as mybir
from concourse import bacc
from concourse.tile import TileContext
from concourse.bass_utils import run_bass_kernel_spmd

BF16 = mybir.dt.bfloat16
F32 = mybir.dt.float32
NP_BF16 = ml_dtypes.bfloat16

WIDTH = 3
H = 128
B = 64
S = 4096
NCORES = 8
BPC = B // NCORES          # sequences per core
XCOLS = S + 2 * WIDTH      # 4102
GROUP = 1024               # tokens per psum block
CHUNK = 512                # matmul free dim
NG = S // GROUP
HALF = 2 * GROUP           # TT-chain quantum

AF = mybir.ActivationFunctionType
ALU = mybir.AluOpType

_CACHE: dict = {}


def _build_nc(relu_mod=13, relu_act=10, pool_sub_mod=0, conv_mod=13, conv_act=3, pool_add_l1=True):
    nc = bacc.Bacc(
        "TRN2",
        target_bir_lowering=False,
        debug=False,
        enable_asserts=True,
        num_devices=NCORES,
    )
    xt = nc.dram_tensor("xt", [BPC, H, XCOLS], BF16, kind="ExternalInput").ap()
    wts = nc.dram_tensor("wts", [H, 14 * H], BF16, kind="ExternalInput").ap()
    bvs = nc.dram_tensor("bvs", [H, 10], F32, kind="ExternalInput").ap()
    out = nc.dram_tensor("out", [BPC, 2, H, S], F32, kind="ExternalOutput").ap()

    state: dict = {}
    relu_i = [0]
    sub_i = [0]
    conv_i = [0]

    with TileContext(nc) as tc:
        with (
            tc.tile_pool(name="const", bufs=1) as const,
            tc.tile_pool(name="xin", bufs=3) as xin,
            tc.tile_pool(name="work", bufs=3) as work,
            tc.tile_pool(name="psum", bufs=1, space="PSUM") as psum,
        ):
            w_sb = const.tile([H, 14 * H], BF16)
            nc.sync.dma_start(out=w_sb, in_=wts)
            b_sb = const.tile([H, 10], F32)
            nc.sync.dma_start(out=b_sb, in_=bvs)

            def highway_a(u, x, l):
                """Layer-l matmuls + relu/sigmoid evacs. Returns (r, gt)."""
                b, side = u
                wi = 6 + side * 4 + l * 2
                bi = 2 + side * 4 + l * 2
                r = work.tile([H, S], BF16, tag="r", name="r", bufs=4)
                gt = work.tile([H, S], BF16, tag="gts", name="gt", bufs=4)
                for half in range(S // HALF):
                    gt_ps = psum.tile([H, HALF], F32, tag="gt", bufs=1,
                                      name="gt_ps")
                    for g2 in range(HALF // GROUP):
                        g = half * (HALF // GROUP) + g2
                        gs = slice(g * GROUP, (g + 1) * GROUP)
                        nl_ps = psum.tile([H, GROUP], F32, tag="ps_a",
                                          bufs=2, name="nl_ps")
                        gh = g2 * GROUP
                        for c in range(GROUP // CHUNK):
                            cs = slice(c * CHUNK, (c + 1) * CHUNK)
                            ps = slice(gh + c * CHUNK, gh + (c + 1) * CHUNK)
                            xs = slice(g * GROUP + c * CHUNK,
                                       g * GROUP + (c + 1) * CHUNK)
                            nc.tensor.matmul(
                                nl_ps[:, cs],
                                w_sb[:, wi * H:(wi + 1) * H],
                                x[:, xs], start=True, stop=True,
                            )
                            nc.tensor.matmul(
                                gt_ps[:, ps],
                                w_sb[:, (wi + 1) * H:(wi + 2) * H],
                                x[:, xs], start=True, stop=True,
                            )
                        # relu evac: split ACT/DVE to balance load
                        if relu_i[0] % relu_mod < relu_act:
                            nc.scalar.activation(
                                r[:, gs], nl_ps, AF.Relu,
                                bias=b_sb[:, bi:bi + 1],
                            )
                        else:
                            nc.vector.tensor_scalar(
                                r[:, gs], nl_ps,
                                b_sb[:, bi:bi + 1], 0.0,
                                op0=ALU.add, op1=ALU.max,
                            )
                        relu_i[0] += 1
                    hs = slice(half * HALF, (half + 1) * HALF)
                    nc.scalar.activation(
                        gt[:, hs], gt_ps, AF.Sigmoid,
                        bias=b_sb[:, bi + 1:bi + 2],
                    )
                return r, gt

            def highway_b(u, x, l, r, gt):
                """Gate combine for layer l. Returns xn."""
                xn = work.tile([H, S], BF16, tag="xn", name="xn")
                for half in range(S // HALF):
                    hs = slice(half * HALF, (half + 1) * HALF)
                    d = work.tile([H, HALF], BF16, tag="d", name="d")
                    nc.vector.tensor_sub(d, x[:, hs], r[:, hs])
                    p = work.tile([H, HALF], BF16, tag="p", name="p")
                    nc.vector.tensor_mul(p, gt[:, hs], d)
                    if l == 1 and pool_add_l1:
                        # final add feeds only the store (also on Pool)
                        nc.gpsimd.tensor_add(xn[:, hs], p, r[:, hs])
                    else:
                        nc.vector.tensor_add(xn[:, hs], p, r[:, hs])
                return xn

            def stage0(u):
                """xt load (once per b) + conv -> x0."""
                b, side = u
                if side == 0:
                    xt_sb = xin.tile([H, XCOLS], BF16, tag="xt", name="xt_sb")
                    nc.sync.dma_start(out=xt_sb, in_=xt[b])
                    state[("xt", b)] = xt_sb
                xt_sb = state[("xt", b)]
                soff = 0 if side == 0 else WIDTH + 1
                x = work.tile([H, S], BF16, tag="x0", name="x0")
                for g in range(NG):
                    conv_ps = psum.tile([H, GROUP], F32, tag="ps_a", bufs=2,
                                        name="conv_ps")
                    for c in range(GROUP // CHUNK):
                        cs = slice(c * CHUNK, (c + 1) * CHUNK)
                        base = g * GROUP + c * CHUNK + soff
                        for i in range(WIDTH):
                            wi = side * 3 + i
                            nc.tensor.matmul(
                                conv_ps[:, cs],
                                w_sb[:, wi * H:(wi + 1) * H],
                                xt_sb[:, base + i: base + i + CHUNK],
                                start=(i == 0), stop=(i == WIDTH - 1),
                            )
                    if conv_i[0] % conv_mod < conv_act:
                        nc.scalar.activation(
                            x[:, g * GROUP:(g + 1) * GROUP], conv_ps,
                            AF.Identity, bias=b_sb[:, side:side + 1],
                        )
                    else:
                        nc.vector.tensor_scalar_add(
                            x[:, g * GROUP:(g + 1) * GROUP], conv_ps,
                            b_sb[:, side:side + 1],
                        )
                    conv_i[0] += 1
                state[("x0", u)] = x

            def stage1(u):
                state[("rg0", u)] = highway_a(u, state[("x0", u)], 0)

            def stage2(u):
                r, gt = state.pop(("rg0", u))
                state[("x1", u)] = highway_b(
                    u, state.pop(("x0", u)), 0, r, gt)

            def stage3(u):
                state[("rg1", u)] = highway_a(u, state[("x1", u)], 1)

            def stage4(u):
                b, side = u
                r, gt = state.pop(("rg1", u))
                x2 = highway_b(u, state.pop(("x1", u)), 1, r, gt)
                nc.gpsimd.dma_start(out=out[b, side], in_=x2)

            units = [(b, side) for b in range(BPC) for side in range(2)]
            n = len(units)
            stages = [stage0, stage1, stage2, stage3, stage4]
            ns = len(stages)
            for k in range(n + ns - 1):
                # oldest unit's stage first: all deps from previous steps
                for s in range(ns - 1, -1, -1):
                    i = k - s
                    if 0 <= i < n:
                        stages[s](units[i])
    nc.compile()
    return nc


def _prep_inputs(inputs):
    """Host-side layout prep: transposed/padded bf16 activations + packed weights."""
    x = np.ascontiguousarray(np.asarray(inputs["inputs"], dtype=np.float32))
    lp = np.asarray(inputs["left_padding"], dtype=np.float32)
    rp = np.asarray(inputs["right_padding"], dtype=np.float32)
    lproj_w = np.asarray(inputs["lproj_w"], dtype=np.float32)
    rproj_w = np.asarray(inputs["rproj_w"], dtype=np.float32)
    lproj_b = np.asarray(inputs["lproj_b"], dtype=np.float32)
    rproj_b = np.asarray(inputs["rproj_b"], dtype=np.float32)
    lhw_w = np.asarray(inputs["lhw_w"], dtype=np.float32)
    rhw_w = np.asarray(inputs["rhw_w"], dtype=np.float32)
    lhw_b = np.asarray(inputs["lhw_b"], dtype=np.float32)
    rhw_b = np.asarray(inputs["rhw_b"], dtype=np.float32)

    xt = np.empty((B, H, XCOLS), NP_BF16)
    xt[:, :, 0:WIDTH] = lp.T.astype(NP_BF16)[None]
    xt[:, :, WIDTH:WIDTH + S] = x.transpose(0, 2, 1).astype(NP_BF16)
    xt[:, :, WIDTH + S:] = rp.T.astype(NP_BF16)[None]

    wts = np.empty((14, H, H), np.float32)
    # conv chunks: W_i[d, h] = proj_w[h, i*128 + d]
    wts[0:3] = lproj_w.reshape(H, WIDTH, H).transpose(1, 2, 0)
    wts[3:6] = rproj_w.reshape(H, WIDTH, H).transpose(1, 2, 0)
    for side, hw in ((0, lhw_w), (1, rhw_w)):
        for l in range(2):
            wts[6 + side * 4 + l * 2] = hw[l, :H, :].T       # nonlinear part
            wts[6 + side * 4 + l * 2 + 1] = hw[l, H:, :].T   # gate part
    # w_sb[d, n*H + h] = wts[n, d, h]
    wts_flat = np.ascontiguousarray(
        wts.transpose(1, 0, 2).reshape(H, 14 * H)
    ).astype(NP_BF16)

    bv = np.zeros((10, H), np.float32)
    bv[0] = lproj_b
    bv[1] = rproj_b
    for side, hb in ((0, lhw_b), (1, rhw_b)):
        for l in range(2):
            bv[2 + side * 4 + l * 2] = hb[l, :H]
            bv[2 + side * 4 + l * 2 + 1] = hb[l, H:]
    bv_t = np.ascontiguousarray(bv.T)  # [128, 10]

    return xt, wts_flat, bv_t


def kernel(**inputs) -> np.ndarray:
    if "nc" not in _CACHE:
        _CACHE["nc"] = _build_nc()
    nc = _CACHE["nc"]

    xt, wts_flat, bv_t = _prep_inputs(inputs)

    in_maps = [
        {
            "xt": np.ascontiguousarray(xt[c * BPC:(c + 1) * BPC]),
            "wts": wts_flat,
            "bvs": bv_t,
        }
        for c in range(NCORES)
    ]
    res = run_bass_kernel_spmd(nc, in_maps, list(range(NCORES))).results

    outp = np.empty((B, S, 2 * H), np.float32)
    for c in range(NCORES):
        o = res[c]["out"]  # [BPC, 2, 128, 4096]
        outp[c * BPC:(c + 1) * BPC] = (
            o.transpose(0, 3, 1, 2).reshape(BPC, S, 2 * H)
        )
    return outp


# revision 12
# speedup vs baseline: 18.2385x; 18.2385x over previous
"""Trainium2 Bass kernel for nn_MLPbiLm (bidirectional conv-window + highway MLP).

Reference computation (eval mode):
  padded = [left_pad(3), x, right_pad(3)]            # per sequence, [S+6, 128]
  left_inp[t]  = padded[t   : t+3]   (tokens t-3..t-1)  -> [384]
  right_inp[t] = padded[t+4 : t+7]   (tokens t+1..t+3)  -> [384]
  left  = highway2(left_inp @ lproj_w.T + lproj_b)
  right = highway2(right_inp @ rproj_w.T + rproj_b)
  out = concat([left, right], -1)                     # [B, S, 256]

Strategy:
  - Data-parallel over batch: 8 sequences per core on 8 NeuronCores.
  - Host prepares x^T in bf16 with padding baked in: xt[b] = [128(d), 4102(t)]
    so the window conv becomes 3 PSUM-accumulated matmuls over *shifted
    column views* of the same SBUF tile (contraction dim d on partitions).
  - All matmuls bf16 (N=512 free dim), PSUM fp32.
  - Work is software-pipelined across 16 units (= 8 seqs x 2 sides) in
    3 stages (conv / highway-0 / highway-1+store) so every engine always
    has independent work queued (engines execute in program order).
  - Elementwise balance: ACT does sigmoid (FD=2048), conv-bias evac and
    ~60% of relu evacs; DVE does the other relus (dual-op tensor_scalar)
    and the gate-combine tensor ops; Pool only does SWDGE cast-stores.
  - Output produced as [b, side, 128(h), 4096(t)] fp32; host transposes
    to [B, S, 256] during unshard.
"""

import numpy as np
import ml_dtypes

import concourse.bass as bass  # noqa: F401
import concourse.mybir as mybir
from concourse import bacc
from concourse.tile import TileContext
from concourse.bass_utils import run_bass_kernel_spmd

BF16 = mybir.dt.bfloat16
F32 = mybir.dt.float32
NP_BF16 = ml_dtypes.bfloat16

WIDTH = 3
H = 128
B = 64
S = 4096
NCORES = 8
BPC = B // NCORES          # sequences per core
XCOLS = S + 2 * WIDTH      # 4102
GROUP = 1024               # tokens per psum block
CHUNK = 512                # matmul free dim
NG = S // GROUP
HALF = 2 * GROUP           # TT-chain quantum

AF = mybir.ActivationFunctionType
ALU = mybir.AluOpType

_CACHE: dict = {}


def _build_nc(relu_mod=13, relu_act=10, pool_sub_mod=0, conv_mod=13, conv_act=3, pool_add_l1=True, work_mult=1):
    nc = bacc.Bacc(
        "TRN2",
        target_bir_lowering=False,
        debug=False,
        enable_asserts=True,
        num_devices=NCORES,
    )
    xt = nc.dram_tensor("xt", [BPC, H, XCOLS], BF16, kind="ExternalInput").ap()
    wts = nc.dram_tensor("wts", [H, 14 * H], BF16, kind="ExternalInput").ap()
    bvs = nc.dram_tensor("bvs", [H, 10], F32, kind="ExternalInput").ap()
    out = nc.dram_tensor("out", [BPC, 2, H, S], F32, kind="ExternalOutput").ap()

    state: dict = {}
    relu_i = [0]
    sub_i = [0]
    conv_i = [0]

    with TileContext(nc) as tc:
        with (
            tc.tile_pool(name="const", bufs=1) as const,
            tc.tile_pool(name="xin", bufs=3) as xin,
            tc.tile_pool(name="work", bufs=3) as work,
            tc.tile_pool(name="psum", bufs=1, space="PSUM") as psum,
        ):
            w_sb = const.tile([H, 14 * H], BF16)
            nc.sync.dma_start(out=w_sb, in_=wts)
            b_sb = const.tile([H, 10], F32)
            nc.sync.dma_start(out=b_sb, in_=bvs)

            def highway_a(u, x, l):
                """Layer-l matmuls + relu/sigmoid evacs. Returns (r, gt)."""
                b, side = u
                wi = 6 + side * 4 + l * 2
                bi = 2 + side * 4 + l * 2
                r = work.tile([H, S], BF16, tag="r", name="r", bufs=4)
                gt = work.tile([H, S], BF16, tag="gts", name="gt", bufs=4)
                for half in range(S // HALF):
                    gt_ps = psum.tile([H, HALF], F32, tag="gt", bufs=1,
                                      name="gt_ps")
                    for g2 in range(HALF // GROUP):
                        g = half * (HALF // GROUP) + g2
                        gs = slice(g * GROUP, (g + 1) * GROUP)
                        nl_ps = psum.tile([H, GROUP], F32, tag="ps_a",
                                          bufs=2, name="nl_ps")
                        gh = g2 * GROUP
                        for c in range(GROUP // CHUNK):
                            cs = slice(c * CHUNK, (c + 1) * CHUNK)
                            ps = slice(gh + c * CHUNK, gh + (c + 1) * CHUNK)
                            xs = slice(g * GROUP + c * CHUNK,
                                       g * GROUP + (c + 1) * CHUNK)
                            nc.tensor.matmul(
                                nl_ps[:, cs],
                                w_sb[:, wi * H:(wi + 1) * H],
                                x[:, xs], start=True, stop=True,
                            )
                            nc.tensor.matmul(
                                gt_ps[:, ps],
                                w_sb[:, (wi + 1) * H:(wi + 2) * H],
                                x[:, xs], start=True, stop=True,
                            )
                        # relu evac: split ACT/DVE to balance load
                        if relu_i[0] % relu_mod < relu_act:
                            nc.scalar.activation(
                                r[:, gs], nl_ps, AF.Relu,
                                bias=b_sb[:, bi:bi + 1],
                            )
                        else:
                            nc.vector.tensor_scalar(
                                r[:, gs], nl_ps,
                                b_sb[:, bi:bi + 1], 0.0,
                                op0=ALU.add, op1=ALU.max,
                            )
                        relu_i[0] += 1
                    hs = slice(half * HALF, (half + 1) * HALF)
                    nc.scalar.activation(
                        gt[:, hs], gt_ps, AF.Sigmoid,
                        bias=b_sb[:, bi + 1:bi + 2],
                    )
                return r, gt

            def highway_b(u, x, l, r, gt):
                """Gate combine for layer l. Returns xn."""
                xn = work.tile([H, S], BF16, tag="xn", name="xn")
                for half in range(S // HALF):
                    hs = slice(half * HALF, (half + 1) * HALF)
                    d = work.tile([H, HALF], BF16, tag="d", name="d")
                    nc.vector.tensor_sub(d, x[:, hs], r[:, hs])
                    p = work.tile([H, HALF], BF16, tag="p", name="p")
                    nc.vector.tensor_mul(p, gt[:, hs], d)
                    if l == 1 and pool_add_l1:
                        # final add feeds only the store (also on Pool)
                        nc.gpsimd.tensor_add(xn[:, hs], p, r[:, hs])
                    else:
                        nc.vector.tensor_add(xn[:, hs], p, r[:, hs])
                return xn

            def stage0(u):
                """xt load (once per b) + conv -> x0."""
                b, side = u
                if side == 0:
                    xt_sb = xin.tile([H, XCOLS], BF16, tag="xt", name="xt_sb")
                    nc.sync.dma_start(out=xt_sb, in_=xt[b])
                    state[("xt", b)] = xt_sb
                xt_sb = state[("xt", b)]
                soff = 0 if side == 0 else WIDTH + 1
                x = work.tile([H, S], BF16, tag="x0", name="x0")
                for g in range(NG):
                    conv_ps = psum.tile([H, GROUP], F32, tag="ps_a", bufs=2,
                                        name="conv_ps")
                    for c in range(GROUP // CHUNK):
                        cs = slice(c * CHUNK, (c + 1) * CHUNK)
                        base = g * GROUP + c * CHUNK + soff
                        for i in range(WIDTH):
                            wi = side * 3 + i
                            nc.tensor.matmul(
                                conv_ps[:, cs],
                                w_sb[:, wi * H:(wi + 1) * H],
                                xt_sb[:, base + i: base + i + CHUNK],
                                start=(i == 0), stop=(i == WIDTH - 1),
                            )
                    if conv_i[0] % conv_mod < conv_act:
                        nc.scalar.activation(
                            x[:, g * GROUP:(g + 1) * GROUP], conv_ps,
                            AF.Identity, bias=b_sb[:, side:side + 1],
                        )
                    else:
                        nc.vector.tensor_scalar_add(
                            x[:, g * GROUP:(g + 1) * GROUP], conv_ps,
                            b_sb[:, side:side + 1],
                        )
                    conv_i[0] += 1
                state[("x0", u)] = x

            def stage1(u):
                state[("rg0", u)] = highway_a(u, state[("x0", u)], 0)

            def stage2(u):
                r, gt = state.pop(("rg0", u))
                state[("x1", u)] = highway_b(
                    u, state.pop(("x0", u)), 0, r, gt)

            def stage3(u):
                state[("rg1", u)] = highway_a(u, state[("x1", u)], 1)

            def stage4(u):
                b, side = u
                r, gt = state.pop(("rg1", u))
                x2 = highway_b(u, state.pop(("x1", u)), 1, r, gt)
                nc.gpsimd.dma_start(out=out[b, side], in_=x2)

            units = [(b, side) for b in range(BPC) for side in range(2)] * work_mult
            n = len(units)
            stages = [stage0, stage1, stage2, stage3, stage4]
            ns = len(stages)
            for k in range(n + ns - 1):
                # oldest unit's stage first: all deps from previous steps
                for s in range(ns - 1, -1, -1):
                    i = k - s
                    if 0 <= i < n:
                        stages[s](units[i])
    nc.compile()
    return nc


def _prep_inputs(inputs):
    """Host-side layout prep: transposed/padded bf16 activations + packed weights."""
    x = np.ascontiguousarray(np.asarray(inputs["inputs"], dtype=np.float32))
    lp = np.asarray(inputs["left_padding"], dtype=np.float32)
    rp = np.asarray(inputs["right_padding"], dtype=np.float32)
    lproj_w = np.asarray(inputs["lproj_w"], dtype=np.float32)
    rproj_w = np.asarray(inputs["rproj_w"], dtype=np.float32)
    lproj_b = np.asarray(inputs["lproj_b"], dtype=np.float32)
    rproj_b = np.asarray(inputs["rproj_b"], dtype=np.float32)
    lhw_w = np.asarray(inputs["lhw_w"], dtype=np.float32)
    rhw_w = np.asarray(inputs["rhw_w"], dtype=np.float32)
    lhw_b = np.asarray(inputs["lhw_b"], dtype=np.float32)
    rhw_b = np.asarray(inputs["rhw_b"], dtype=np.float32)

    xt = np.empty((B, H, XCOLS), NP_BF16)
    xt[:, :, 0:WIDTH] = lp.T.astype(NP_BF16)[None]
    xt[:, :, WIDTH:WIDTH + S] = x.transpose(0, 2, 1).astype(NP_BF16)
    xt[:, :, WIDTH + S:] = rp.T.astype(NP_BF16)[None]

    wts = np.empty((14, H, H), np.float32)
    # conv chunks: W_i[d, h] = proj_w[h, i*128 + d]
    wts[0:3] = lproj_w.reshape(H, WIDTH, H).transpose(1, 2, 0)
    wts[3:6] = rproj_w.reshape(H, WIDTH, H).transpose(1, 2, 0)
    for side, hw in ((0, lhw_w), (1, rhw_w)):
        for l in range(2):
            wts[6 + side * 4 + l * 2] = hw[l, :H, :].T       # nonlinear part
            wts[6 + side * 4 + l * 2 + 1] = hw[l, H:, :].T   # gate part
    # w_sb[d, n*H + h] = wts[n, d, h]
    wts_flat = np.ascontiguousarray(
        wts.transpose(1, 0, 2).reshape(H, 14 * H)
    ).astype(NP_BF16)

    bv = np.zeros((10, H), np.float32)
    bv[0] = lproj_b
    bv[1] = rproj_b
    for side, hb in ((0, lhw_b), (1, rhw_b)):
        for l in range(2):
            bv[2 + side * 4 + l * 2] = hb[l, :H]
            bv[2 + side * 4 + l * 2 + 1] = hb[l, H:]
    bv_t = np.ascontiguousarray(bv.T)  # [128, 10]

    return xt, wts_flat, bv_t


def kernel(**inputs) -> np.ndarray:
    if "nc" not in _CACHE:
        _CACHE["nc"] = _build_nc()
    nc = _CACHE["nc"]

    xt, wts_flat, bv_t = _prep_inputs(inputs)

    in_maps = [
        {
            "xt": np.ascontiguousarray(xt[c * BPC:(c + 1) * BPC]),
            "wts": wts_flat,
            "bvs": bv_t,
        }
        for c in range(NCORES)
    ]
    res = run_bass_kernel_spmd(nc, in_maps, list(range(NCORES))).results

    outp = np.empty((B, S, 2 * H), np.float32)
    for c in range(NCORES):
        o = res[c]["out"]  # [BPC, 2, 128, 4096]
        outp[c * BPC:(c + 1) * BPC] = (
            o.transpose(0, 3, 1, 2).reshape(BPC, S, 2 * H)
        )
    return outp


# revision 15
# speedup vs baseline: 18.6127x; 1.0205x over previous
"""Trainium2 Bass kernel for nn_MLPbiLm (bidirectional conv-window + highway MLP).

Reference computation (eval mode):
  padded = [left_pad(3), x, right_pad(3)]            # per sequence, [S+6, 128]
  left_inp[t]  = padded[t   : t+3]   (tokens t-3..t-1)  -> [384]
  right_inp[t] = padded[t+4 : t+7]   (tokens t+1..t+3)  -> [384]
  left  = highway2(left_inp @ lproj_w.T + lproj_b)
  right = highway2(right_inp @ rproj_w.T + rproj_b)
  out = concat([left, right], -1)                     # [B, S, 256]

Strategy:
  - Data-parallel over batch: 8 sequences per core on 8 NeuronCores.
  - Host prepares x^T in bf16 with padding baked in: xt[b] = [128(d), 4102(t)]
    so the window conv becomes 3 PSUM-accumulated matmuls over *shifted
    column views* of the same SBUF tile (contraction dim d on partitions).
  - All matmuls bf16 (N=512 free dim), PSUM fp32.
  - Work is software-pipelined across 16 units (= 8 seqs x 2 sides) in
    3 stages (conv / highway-0 / highway-1+store) so every engine always
    has independent work queued (engines execute in program order).
  - Elementwise balance: ACT does sigmoid (FD=2048), conv-bias evac and
    ~60% of relu evacs; DVE does the other relus (dual-op tensor_scalar)
    and the gate-combine tensor ops; Pool only does SWDGE cast-stores.
  - Output produced as [b, side, 128(h), 4096(t)] fp32; host transposes
    to [B, S, 256] during unshard.
"""

import numpy as np
import ml_dtypes

import concourse.bass as bass  # noqa: F401
import concourse.mybir as mybir
from concourse import bacc
from concourse.tile import TileContext
from concourse.bass_utils import run_bass_kernel_spmd

BF16 = mybir.dt.bfloat16
F32 = mybir.dt.float32
NP_BF16 = ml_dtypes.bfloat16

WIDTH = 3
H = 128
B = 64
S = 4096
NCORES = 8
BPC = B // NCORES          # sequences per core
XCOLS = S + 2 * WIDTH      # 4102
GROUP = 1024               # tokens per psum block
CHUNK = 512                # matmul free dim
NG = S // GROUP
HALF = 2 * GROUP           # TT-chain quantum

AF = mybir.ActivationFunctionType
ALU = mybir.AluOpType

_CACHE: dict = {}


def _build_nc(relu_mod=5, relu_act=4, pool_sub_mod=0, conv_mod=13, conv_act=2, pool_add_l1=True, work_mult=1, chain_q=HALF, split_store=False, x_bufs=3):
    nc = bacc.Bacc(
        "TRN2",
        target_bir_lowering=False,
        debug=False,
        enable_asserts=True,
        num_devices=NCORES,
    )
    xt = nc.dram_tensor("xt", [BPC, H, XCOLS], BF16, kind="ExternalInput").ap()
    wts = nc.dram_tensor("wts", [H, 14 * H], BF16, kind="ExternalInput").ap()
    bvs = nc.dram_tensor("bvs", [H, 10], F32, kind="ExternalInput").ap()
    out = nc.dram_tensor("out", [BPC, 2, H, S], F32, kind="ExternalOutput").ap()

    state: dict = {}
    relu_i = [0]
    sub_i = [0]
    conv_i = [0]

    with TileContext(nc) as tc:
        with (
            tc.tile_pool(name="const", bufs=1) as const,
            tc.tile_pool(name="xin", bufs=3) as xin,
            tc.tile_pool(name="work", bufs=3) as work,
            tc.tile_pool(name="psum", bufs=1, space="PSUM") as psum,
        ):
            w_sb = const.tile([H, 14 * H], BF16)
            nc.sync.dma_start(out=w_sb, in_=wts)
            b_sb = const.tile([H, 10], F32)
            nc.sync.dma_start(out=b_sb, in_=bvs)

            def highway_a(u, x, l):
                """Layer-l matmuls + relu/sigmoid evacs. Returns (r, gt)."""
                b, side = u
                wi = 6 + side * 4 + l * 2
                bi = 2 + side * 4 + l * 2
                r = work.tile([H, S], BF16, tag="r", name="r", bufs=4)
                gt = work.tile([H, S], BF16, tag="gts", name="gt", bufs=4)
                for half in range(S // HALF):
                    gt_ps = psum.tile([H, HALF], F32, tag="gt", bufs=1,
                                      name="gt_ps")
                    for g2 in range(HALF // GROUP):
                        g = half * (HALF // GROUP) + g2
                        gs = slice(g * GROUP, (g + 1) * GROUP)
                        nl_ps = psum.tile([H, GROUP], F32, tag="ps_a",
                                          bufs=2, name="nl_ps")
                        gh = g2 * GROUP
                        for c in range(GROUP // CHUNK):
                            cs = slice(c * CHUNK, (c + 1) * CHUNK)
                            ps = slice(gh + c * CHUNK, gh + (c + 1) * CHUNK)
                            xs = slice(g * GROUP + c * CHUNK,
                                       g * GROUP + (c + 1) * CHUNK)
                            nc.tensor.matmul(
                                nl_ps[:, cs],
                                w_sb[:, wi * H:(wi + 1) * H],
                                x[:, xs], start=True, stop=True,
                            )
                            nc.tensor.matmul(
                                gt_ps[:, ps],
                                w_sb[:, (wi + 1) * H:(wi + 2) * H],
                                x[:, xs], start=True, stop=True,
                            )
                        # relu evac: split ACT/DVE to balance load
                        if relu_i[0] % relu_mod < relu_act:
                            nc.scalar.activation(
                                r[:, gs], nl_ps, AF.Relu,
                                bias=b_sb[:, bi:bi + 1],
                            )
                        else:
                            nc.vector.tensor_scalar(
                                r[:, gs], nl_ps,
                                b_sb[:, bi:bi + 1], 0.0,
                                op0=ALU.add, op1=ALU.max,
                            )
                        relu_i[0] += 1
                    hs = slice(half * HALF, (half + 1) * HALF)
                    nc.scalar.activation(
                        gt[:, hs], gt_ps, AF.Sigmoid,
                        bias=b_sb[:, bi + 1:bi + 2],
                    )
                return r, gt

            def highway_b(u, x, l, r, gt, store_to=None):
                """Gate combine for layer l. Returns xn."""
                xn = work.tile([H, S], BF16, tag="xn", name="xn", bufs=x_bufs)
                for half in range(S // chain_q):
                    hs = slice(half * chain_q, (half + 1) * chain_q)
                    d = work.tile([H, chain_q], BF16, tag="d", name="d")
                    nc.vector.tensor_sub(d, x[:, hs], r[:, hs])
                    p = work.tile([H, chain_q], BF16, tag="p", name="p")
                    nc.vector.tensor_mul(p, gt[:, hs], d)
                    if l == 1 and pool_add_l1:
                        # final add feeds only the store (also on Pool)
                        nc.gpsimd.tensor_add(xn[:, hs], p, r[:, hs])
                    else:
                        nc.vector.tensor_add(xn[:, hs], p, r[:, hs])
                    if store_to is not None and split_store:
                        nc.gpsimd.dma_start(out=store_to[:, hs], in_=xn[:, hs])
                if store_to is not None and not split_store:
                    nc.gpsimd.dma_start(out=store_to, in_=xn)
                return xn

            def stage0(u):
                """xt load (once per b) + conv -> x0."""
                b, side = u
                if side == 0:
                    xt_sb = xin.tile([H, XCOLS], BF16, tag="xt", name="xt_sb")
                    nc.sync.dma_start(out=xt_sb, in_=xt[b])
                    state[("xt", b)] = xt_sb
                xt_sb = state[("xt", b)]
                soff = 0 if side == 0 else WIDTH + 1
                x = work.tile([H, S], BF16, tag="x0", name="x0", bufs=x_bufs)
                for g in range(NG):
                    conv_ps = psum.tile([H, GROUP], F32, tag="ps_a", bufs=2,
                                        name="conv_ps")
                    for c in range(GROUP // CHUNK):
                        cs = slice(c * CHUNK, (c + 1) * CHUNK)
                        base = g * GROUP + c * CHUNK + soff
                        for i in range(WIDTH):
                            wi = side * 3 + i
                            nc.tensor.matmul(
                                conv_ps[:, cs],
                                w_sb[:, wi * H:(wi + 1) * H],
                                xt_sb[:, base + i: base + i + CHUNK],
                                start=(i == 0), stop=(i == WIDTH - 1),
                            )
                    if conv_i[0] % conv_mod < conv_act:
                        nc.scalar.activation(
                            x[:, g * GROUP:(g + 1) * GROUP], conv_ps,
                            AF.Identity, bias=b_sb[:, side:side + 1],
                        )
                    else:
                        nc.vector.tensor_scalar_add(
                            x[:, g * GROUP:(g + 1) * GROUP], conv_ps,
                            b_sb[:, side:side + 1],
                        )
                    conv_i[0] += 1
                state[("x0", u)] = x

            def stage1(u):
                state[("rg0", u)] = highway_a(u, state[("x0", u)], 0)

            def stage2(u):
                r, gt = state.pop(("rg0", u))
                state[("x1", u)] = highway_b(
                    u, state.pop(("x0", u)), 0, r, gt)

            def stage3(u):
                state[("rg1", u)] = highway_a(u, state[("x1", u)], 1)

            def stage4(u):
                b, side = u
                r, gt = state.pop(("rg1", u))
                highway_b(u, state.pop(("x1", u)), 1, r, gt,
                          store_to=out[b, side])

            units = [(b, side) for b in range(BPC) for side in range(2)] * work_mult
            n = len(units)
            stages = [stage0, stage1, stage2, stage3, stage4]
            ns = len(stages)
            for k in range(n + ns - 1):
                # oldest unit's stage first: all deps from previous steps
                for s in range(ns - 1, -1, -1):
                    i = k - s
                    if 0 <= i < n:
                        stages[s](units[i])
    nc.compile()
    return nc


def _prep_inputs(inputs):
    """Host-side layout prep: transposed/padded bf16 activations + packed weights."""
    x = np.ascontiguousarray(np.asarray(inputs["inputs"], dtype=np.float32))
    lp = np.asarray(inputs["left_padding"], dtype=np.float32)
    rp = np.asarray(inputs["right_padding"], dtype=np.float32)
    lproj_w = np.asarray(inputs["lproj_w"], dtype=np.float32)
    rproj_w = np.asarray(inputs["rproj_w"], dtype=np.float32)
    lproj_b = np.asarray(inputs["lproj_b"], dtype=np.float32)
    rproj_b = np.asarray(inputs["rproj_b"], dtype=np.float32)
    lhw_w = np.asarray(inputs["lhw_w"], dtype=np.float32)
    rhw_w = np.asarray(inputs["rhw_w"], dtype=np.float32)
    lhw_b = np.asarray(inputs["lhw_b"], dtype=np.float32)
    rhw_b = np.asarray(inputs["rhw_b"], dtype=np.float32)

    xt = np.empty((B, H, XCOLS), NP_BF16)
    xt[:, :, 0:WIDTH] = lp.T.astype(NP_BF16)[None]
    xt[:, :, WIDTH:WIDTH + S] = x.transpose(0, 2, 1).astype(NP_BF16)
    xt[:, :, WIDTH + S:] = rp.T.astype(NP_BF16)[None]

    wts = np.empty((14, H, H), np.float32)
    # conv chunks: W_i[d, h] = proj_w[h, i*128 + d]
    wts[0:3] = lproj_w.reshape(H, WIDTH, H).transpose(1, 2, 0)
    wts[3:6] = rproj_w.reshape(H, WIDTH, H).transpose(1, 2, 0)
    for side, hw in ((0, lhw_w), (1, rhw_w)):
        for l in range(2):
            wts[6 + side * 4 + l * 2] = hw[l, :H, :].T       # nonlinear part
            wts[6 + side * 4 + l * 2 + 1] = hw[l, H:, :].T   # gate part
    # w_sb[d, n*H + h] = wts[n, d, h]
    wts_flat = np.ascontiguousarray(
        wts.transpose(1, 0, 2).reshape(H, 14 * H)
    ).astype(NP_BF16)

    bv = np.zeros((10, H), np.float32)
    bv[0] = lproj_b
    bv[1] = rproj_b
    for side, hb in ((0, lhw_b), (1, rhw_b)):
        for l in range(2):
            bv[2 + side * 4 + l * 2] = hb[l, :H]
            bv[2 + side * 4 + l * 2 + 1] = hb[l, H:]
    bv_t = np.ascontiguousarray(bv.T)  # [128, 10]

    return xt, wts_flat, bv_t


def kernel(**inputs) -> np.ndarray:
    if "nc" not in _CACHE:
        _CACHE["nc"] = _build_nc()
    nc = _CACHE["nc"]

    xt, wts_flat, bv_t = _prep_inputs(inputs)

    in_maps = [
        {
            "xt": np.ascontiguousarray(xt[c * BPC:(c + 1) * BPC]),
            "wts": wts_flat,
            "bvs": bv_t,
        }
        for c in range(NCORES)
    ]
    res = run_bass_kernel_spmd(nc, in_maps, list(range(NCORES))).results

    outp = np.empty((B, S, 2 * H), np.float32)
    for c in range(NCORES):
        o = res[c]["out"]  # [BPC, 2, 128, 4096]
        outp[c * BPC:(c + 1) * BPC] = (
            o.transpose(0, 3, 1, 2).reshape(BPC, S, 2 * H)
        )
    return outp


# revision 19
# speedup vs baseline: 19.1434x; 1.0285x over previous
"""Trainium2 Bass kernel for nn_MLPbiLm (bidirectional conv-window + highway MLP).

Reference computation (eval mode):
  padded = [left_pad(3), x, right_pad(3)]            # per sequence, [S+6, 128]
  left_inp[t]  = padded[t   : t+3]   (tokens t-3..t-1)  -> [384]
  right_inp[t] = padded[t+4 : t+7]   (tokens t+1..t+3)  -> [384]
  left  = highway2(left_inp @ lproj_w.T + lproj_b)
  right = highway2(right_inp @ rproj_w.T + rproj_b)
  out = concat([left, right], -1)                     # [B, S, 256]

Strategy:
  - Data-parallel over batch: 8 sequences per core on 8 NeuronCores.
  - Host prepares x^T in bf16 with padding baked in: xt[b] = [128(d), 4102(t)]
    so the window conv becomes 3 PSUM-accumulated matmuls over *shifted
    column views* of the same SBUF tile (contraction dim d on partitions).
  - All matmuls bf16 (N=512 free dim), PSUM fp32.
  - Work is software-pipelined across 16 units (= 8 seqs x 2 sides) in
    3 stages (conv / highway-0 / highway-1+store) so every engine always
    has independent work queued (engines execute in program order).
  - Elementwise balance: ACT does sigmoid (FD=2048), conv-bias evac and
    ~60% of relu evacs; DVE does the other relus (dual-op tensor_scalar)
    and the gate-combine tensor ops; Pool only does SWDGE cast-stores.
  - Output produced as [b, side, 128(h), 4096(t)] fp32; host transposes
    to [B, S, 256] during unshard.
"""

import numpy as np
import ml_dtypes

import concourse.bass as bass  # noqa: F401
import concourse.mybir as mybir
from concourse import bacc
from concourse.tile import TileContext
from concourse.bass_utils import run_bass_kernel_spmd

BF16 = mybir.dt.bfloat16
F32 = mybir.dt.float32
NP_BF16 = ml_dtypes.bfloat16

WIDTH = 3
H = 128
B = 64
S = 4096
NCORES = 8
BPC = B // NCORES          # sequences per core
XCOLS = S + 2 * WIDTH      # 4102
GROUP = 1024               # tokens per psum block
CHUNK = 512                # matmul free dim
NG = S // GROUP
HALF = 2 * GROUP           # TT-chain quantum

AF = mybir.ActivationFunctionType
ALU = mybir.AluOpType

_CACHE: dict = {}


def _build_nc(relu_mod=7, relu_act=6, pool_sub_mod=0, conv_mod=13, conv_act=1, pool_add_l1=True, work_mult=1, chain_q=HALF, split_store=False, x_bufs=3, relu_dve_gs=None, rg_bufs=4, nsub=2):
    nc = bacc.Bacc(
        "TRN2",
        target_bir_lowering=False,
        debug=False,
        enable_asserts=True,
        num_devices=NCORES,
    )
    xt = nc.dram_tensor("xt", [BPC, H, XCOLS], BF16, kind="ExternalInput").ap()
    wts = nc.dram_tensor("wts", [H, 14 * H], BF16, kind="ExternalInput").ap()
    bvs = nc.dram_tensor("bvs", [H, 10], F32, kind="ExternalInput").ap()
    out = nc.dram_tensor("out", [BPC, 2, H, S], F32, kind="ExternalOutput").ap()

    SUB = S // nsub
    state: dict = {}
    relu_i = [0]
    sub_i = [0]
    conv_i = [0]

    with TileContext(nc) as tc:
        with (
            tc.tile_pool(name="const", bufs=1) as const,
            tc.tile_pool(name="xin", bufs=3) as xin,
            tc.tile_pool(name="work", bufs=3) as work,
            tc.tile_pool(name="psum", bufs=1, space="PSUM") as psum,
        ):
            w_sb = const.tile([H, 14 * H], BF16)
            nc.sync.dma_start(out=w_sb, in_=wts)
            b_sb = const.tile([H, 10], F32)
            nc.sync.dma_start(out=b_sb, in_=bvs)

            def highway_a(u, x, l):
                """Layer-l matmuls + relu/sigmoid evacs over the subunit's
                SUB tokens (x is a [H, SUB] tile). Returns (r, gt)."""
                b, side, h0 = u
                wi = 6 + side * 4 + l * 2
                bi = 2 + side * 4 + l * 2
                r = work.tile([H, SUB], BF16, tag="r", name="r", bufs=rg_bufs)
                gt = work.tile([H, SUB], BF16, tag="gts", name="gt",
                               bufs=rg_bufs)
                for half in range(SUB // HALF):
                    gt_ps = psum.tile([H, HALF], F32, tag="gt", bufs=1,
                                      name="gt_ps")
                    for g2 in range(HALF // GROUP):
                        g = half * (HALF // GROUP) + g2
                        gs = slice(g * GROUP, (g + 1) * GROUP)
                        nl_ps = psum.tile([H, GROUP], F32, tag="ps_a",
                                          bufs=2, name="nl_ps")
                        gh = g2 * GROUP
                        for c in range(GROUP // CHUNK):
                            cs = slice(c * CHUNK, (c + 1) * CHUNK)
                            ps = slice(gh + c * CHUNK, gh + (c + 1) * CHUNK)
                            xs = slice(g * GROUP + c * CHUNK,
                                       g * GROUP + (c + 1) * CHUNK)
                            nc.tensor.matmul(
                                nl_ps[:, cs],
                                w_sb[:, wi * H:(wi + 1) * H],
                                x[:, xs], start=True, stop=True,
                            )
                            nc.tensor.matmul(
                                gt_ps[:, ps],
                                w_sb[:, (wi + 1) * H:(wi + 2) * H],
                                x[:, xs], start=True, stop=True,
                            )
                        # relu evac: split ACT/DVE to balance load
                        if relu_i[0] % relu_mod < relu_act:
                            nc.scalar.activation(
                                r[:, gs], nl_ps, AF.Relu,
                                bias=b_sb[:, bi:bi + 1],
                            )
                        else:
                            nc.vector.tensor_scalar(
                                r[:, gs], nl_ps,
                                b_sb[:, bi:bi + 1], 0.0,
                                op0=ALU.add, op1=ALU.max,
                            )
                        relu_i[0] += 1
                    hs = slice(half * HALF, (half + 1) * HALF)
                    nc.scalar.activation(
                        gt[:, hs], gt_ps, AF.Sigmoid,
                        bias=b_sb[:, bi + 1:bi + 2],
                    )
                return r, gt

            def highway_b(u, x, l, r, gt, store_to=None):
                """Gate combine for layer l of one subunit. Returns xn."""
                b, side, h0 = u
                xn = work.tile([H, SUB], BF16, tag="xn", name="xn",
                               bufs=x_bufs)
                for half in range(SUB // chain_q):
                    hs = slice(half * chain_q, (half + 1) * chain_q)
                    d = work.tile([H, chain_q], BF16, tag="d", name="d")
                    nc.vector.tensor_sub(d, x[:, hs], r[:, hs])
                    p = work.tile([H, chain_q], BF16, tag="p", name="p")
                    nc.vector.tensor_mul(p, gt[:, hs], d)
                    if l == 1 and pool_add_l1:
                        # final add feeds only the store (also on Pool)
                        nc.gpsimd.tensor_add(xn[:, hs], p, r[:, hs])
                    else:
                        nc.vector.tensor_add(xn[:, hs], p, r[:, hs])
                if store_to is not None:
                    nc.gpsimd.dma_start(out=store_to, in_=xn)
                return xn

            def stage0(u):
                """xt load (once per b) + conv -> x0 for this subunit."""
                b, side, h0 = u
                if side == 0 and h0 == 0:
                    xt_sb = xin.tile([H, XCOLS], BF16, tag="xt", name="xt_sb")
                    nc.sync.dma_start(out=xt_sb, in_=xt[b])
                    state[("xt", b)] = xt_sb
                xt_sb = state[("xt", b)]
                soff = (0 if side == 0 else WIDTH + 1) + h0 * SUB
                x = work.tile([H, SUB], BF16, tag="x0", name="x0", bufs=x_bufs)
                for g in range(SUB // GROUP):
                    conv_ps = psum.tile([H, GROUP], F32, tag="ps_a", bufs=2,
                                        name="conv_ps")
                    for c in range(GROUP // CHUNK):
                        cs = slice(c * CHUNK, (c + 1) * CHUNK)
                        base = g * GROUP + c * CHUNK + soff
                        for i in range(WIDTH):
                            wi = side * 3 + i
                            nc.tensor.matmul(
                                conv_ps[:, cs],
                                w_sb[:, wi * H:(wi + 1) * H],
                                xt_sb[:, base + i: base + i + CHUNK],
                                start=(i == 0), stop=(i == WIDTH - 1),
                            )
                    if conv_i[0] % conv_mod < conv_act:
                        nc.scalar.activation(
                            x[:, g * GROUP:(g + 1) * GROUP], conv_ps,
                            AF.Identity, bias=b_sb[:, side:side + 1],
                        )
                    else:
                        nc.vector.tensor_scalar_add(
                            x[:, g * GROUP:(g + 1) * GROUP], conv_ps,
                            b_sb[:, side:side + 1],
                        )
                    conv_i[0] += 1
                state[("x0", u)] = x

            def stage1(u):
                state[("rg0", u)] = highway_a(u, state[("x0", u)], 0)

            def stage2(u):
                r, gt = state.pop(("rg0", u))
                state[("x1", u)] = highway_b(
                    u, state.pop(("x0", u)), 0, r, gt)

            def stage3(u):
                state[("rg1", u)] = highway_a(u, state[("x1", u)], 1)

            def stage4(u):
                b, side, h0 = u
                r, gt = state.pop(("rg1", u))
                highway_b(u, state.pop(("x1", u)), 1, r, gt,
                          store_to=out[b, side, :, h0 * SUB:(h0 + 1) * SUB])

            units = [(b, side, h0)
                     for b in range(BPC) for side in range(2)
                     for h0 in range(nsub)] * work_mult
            n = len(units)
            stages = [stage0, stage1, stage2, stage3, stage4]
            ns = len(stages)
            for k in range(n + ns - 1):
                # oldest unit's stage first: all deps from previous steps
                for s in range(ns - 1, -1, -1):
                    i = k - s
                    if 0 <= i < n:
                        stages[s](units[i])
    nc.compile()
    return nc


def _prep_inputs(inputs):
    """Host-side layout prep: transposed/padded bf16 activations + packed weights."""
    x = np.ascontiguousarray(np.asarray(inputs["inputs"], dtype=np.float32))
    lp = np.asarray(inputs["left_padding"], dtype=np.float32)
    rp = np.asarray(inputs["right_padding"], dtype=np.float32)
    lproj_w = np.asarray(inputs["lproj_w"], dtype=np.float32)
    rproj_w = np.asarray(inputs["rproj_w"], dtype=np.float32)
    lproj_b = np.asarray(inputs["lproj_b"], dtype=np.float32)
    rproj_b = np.asarray(inputs["rproj_b"], dtype=np.float32)
    lhw_w = np.asarray(inputs["lhw_w"], dtype=np.float32)
    rhw_w = np.asarray(inputs["rhw_w"], dtype=np.float32)
    lhw_b = np.asarray(inputs["lhw_b"], dtype=np.float32)
    rhw_b = np.asarray(inputs["rhw_b"], dtype=np.float32)

    xt = np.empty((B, H, XCOLS), NP_BF16)
    xt[:, :, 0:WIDTH] = lp.T.astype(NP_BF16)[None]
    xt[:, :, WIDTH:WIDTH + S] = x.transpose(0, 2, 1).astype(NP_BF16)
    xt[:, :, WIDTH + S:] = rp.T.astype(NP_BF16)[None]

    wts = np.empty((14, H, H), np.float32)
    # conv chunks: W_i[d, h] = proj_w[h, i*128 + d]
    wts[0:3] = lproj_w.reshape(H, WIDTH, H).transpose(1, 2, 0)
    wts[3:6] = rproj_w.reshape(H, WIDTH, H).transpose(1, 2, 0)
    for side, hw in ((0, lhw_w), (1, rhw_w)):
        for l in range(2):
            wts[6 + side * 4 + l * 2] = hw[l, :H, :].T       # nonlinear part
            wts[6 + side * 4 + l * 2 + 1] = hw[l, H:, :].T   # gate part
    # w_sb[d, n*H + h] = wts[n, d, h]
    wts_flat = np.ascontiguousarray(
        wts.transpose(1, 0, 2).reshape(H, 14 * H)
    ).astype(NP_BF16)

    bv = np.zeros((10, H), np.float32)
    bv[0] = lproj_b
    bv[1] = rproj_b
    for side, hb in ((0, lhw_b), (1, rhw_b)):
        for l in range(2):
            bv[2 + side * 4 + l * 2] = hb[l, :H]
            bv[2 + side * 4 + l * 2 + 1] = hb[l, H:]
    bv_t = np.ascontiguousarray(bv.T)  # [128, 10]

    return xt, wts_flat, bv_t


def kernel(**inputs) -> np.ndarray:
    if "nc" not in _CACHE:
        _CACHE["nc"] = _build_nc()
    nc = _CACHE["nc"]

    xt, wts_flat, bv_t = _prep_inputs(inputs)

    in_maps = [
        {
            "xt": np.ascontiguousarray(xt[c * BPC:(c + 1) * BPC]),
            "wts": wts_flat,
            "bvs": bv_t,
        }
        for c in range(NCORES)
    ]
    res = run_bass_kernel_spmd(nc, in_maps, list(range(NCORES))).results

    outp = np.empty((B, S, 2 * H), np.float32)
    for c in range(NCORES):
        o = res[c]["out"]  # [BPC, 2, 128, 4096]
        outp[c * BPC:(c + 1) * BPC] = (
            o.transpose(0, 3, 1, 2).reshape(BPC, S, 2 * H)
        )
    return outp
